# revision 30
# baseline (speedup 1.0000x reference)
"""Trainium2 Bass kernel for nn_G3DCrossAttention (B=2, C=512, L=2048, G=2048, H=8).

Exact-math rank-1 collapse of the attention (see kernel_v1_baseline.py for the
derivation): exp_p is rank-1 in channels, so per head the attention output is
x_attn = w*u_v + c_v with w = f_b(a), a = x_seq @ M + a0. f_b is evaluated at
64 Chebyshev nodes on device (exact softmax-collapse over all G genes), fit
with a KDEG-term Chebyshev series and evaluated by a Clenshaw recurrence.

v3 structure (vs the 175us baseline):
  - u_k/u_v/c_v/M/a0 depend only on weights -> precomputed host-side in numpy
    and shipped as packed constants (kills the 38us on-device stage A and 3MB
    of Wq/Wk/Wv DMA traffic)
  - e_b node matrix built by a K=2 block-ones matmul from a [2,G] tile instead
    of a broadcast DMA (whose descriptor generation took 21us to issue)
  - Chebyshev coefficients broadcast by a block-ones matmul (no DRAM trip)
  - per-head w broadcast by a selector matmul from a [H,T] tile; c_v folded in
    as a K=1 matmul; the [H,T] tile comes from a 2-DMA SBUF->SBUF repack
  - LN rstd via ACT Abs_reciprocal_sqrt (40000-bucket table; one table switch
    total) instead of single-lane reciprocal (3.3us) or Ln+Exp (table thrash)
  - all constants packed into a handful of DMAs; fp16 weights one DMA each
  - KDEG=16 (w err ~4e-4; full-pipeline fp32 err 2.5e-4; gate is 2e-2)

Sharding: data-parallel over L across 8 cores (L/8 = 256 queries each).
"""

from contextlib import ExitStack

import numpy as np

import concourse.bass as bass
import concourse.tile as tile
from concourse import bacc, mybir
from concourse.bass_utils import run_bass_kernel_spmd

F32 = mybir.dt.float32
F32R = mybir.dt.float32r
FP16 = mybir.dt.float16
AF = mybir.ActivationFunctionType
OP = mybir.AluOpType
AX = mybir.AxisListType

B, C, L, G, H = 2, 512, 2048, 2048, 8
D = C // H
NCORES = 8
LC = L // NCORES              # 256 queries per core
T = B * LC                    # 512 tokens per core (tau = b*LC + l)
KC = C // 128                 # 4 partition tiles over C
KH = (4 * C) // 128           # 16 partition tiles over 4C
FP = LC // 8                  # 32: free dim of the packed a/w tiles
GC = G // 512                 # 4 chunks over genes
SCALE = 1.0 / float(np.sqrt(D))
EPS = 1e-5
SCAL = 5.0                    # Chebyshev half-range in a-units (|a|max ~ 4.43)
KDEG = 16                     # Chebyshev series length
MNODES = 64                   # Chebyshev nodes per batch (2 batches -> 128 parts)
NPC = 7                       # per-kt cols: boP uv g1 onesC cv2C miscA miscB

TRACE = False
TRACE_KW = {}
LAST_RESULTS = None
DBG = False

_CACHE = None


def _consts():
    m = np.arange(MNODES)
    theta = np.pi * (2 * m + 1) / (2 * MNODES)
    xn64 = (SCAL * np.cos(theta)).astype(np.float32)
    xnodes = np.concatenate([xn64, xn64])                 # [128] both batches
    dct1 = np.zeros((MNODES, KDEG), np.float32)
    for k in range(KDEG):
        dct1[:, k] = (2.0 / MNODES) * np.cos(k * theta)
    dct1[:, 0] *= 0.5
    dct_full = np.concatenate([dct1, dct1], axis=0)       # [128, KDEG]
    blockones = np.zeros((128, 128), np.float32)
    blockones[:64, :64] = 1.0
    blockones[64:, 64:] = 1.0
    # reverse repack selector: w_pack [128,FP] -> w_HT [8,T]
    # c_rv[q, (b*8+lhi)*8 + h] = 1 iff q == b*64 + h*8 + lhi
    rv = np.zeros((128, 128), np.float32)
    for b in range(2):
        for lhi in range(8):
            for h in range(H):
                rv[b * 64 + h * 8 + lhi, (b * 8 + lhi) * 8 + h] = 1.0
    # cst layout: [dct KDEG][xn 1][blockones 128][rv 128]
    cst = np.concatenate([dct_full, xnodes[:, None], blockones, rv], axis=1)
    sel = np.zeros((H, C), np.float32)                    # sel[h, c] = [c//D == h]
    for h in range(H):
        sel[h, h * D:(h + 1) * D] = 1.0
    # forward repack selector: tt_sb [8,T] -> tt [128,FP]
    # rp[j, lhi*64 + p] = 1 iff p == j*8 + lhi   (per batch-half)
    rp = np.zeros((H, 512), np.float32)
    for lhi in range(8):
        for j in range(H):
            rp[j, lhi * 64 + j * 8 + lhi] = 1.0
    sel2 = np.concatenate([sel, rp], axis=1)              # [8, C + 512]
    bh = np.zeros((2, 128), np.float32)                   # batch-half selector
    bh[0, :64] = 1.0
    bh[1, 64:] = 1.0
    return cst, sel2, bh


def _build():
    nc = bacc.Bacc(debug=False, num_devices=NCORES)

    # ---- external inputs -------------------------------------------------
    seq_sl = nc.dram_tensor("seq_sl", [128, KC * T], F32, kind="ExternalInput")
    expv = nc.dram_tensor("expv", [B, G], F32, kind="ExternalInput")
    w1t = nc.dram_tensor("w1t", [128, KC * 4 * C], FP16, kind="ExternalInput")
    w2t = nc.dram_tensor("w2t", [128, KH * C], FP16, kind="ExternalInput")
    wot = nc.dram_tensor("wot", [128, KC * C], FP16, kind="ExternalInput")
    pcd = nc.dram_tensor("pcd", [128, KC * NPC], F32, kind="ExternalInput")
    md = nc.dram_tensor("md", [128, KC * H], F32, kind="ExternalInput")  # M tiles
    vd = nc.dram_tensor("vd", [128, KC * H], F32, kind="ExternalInput")  # V tiles
    b1d = nc.dram_tensor("b1d", [128, KH], F32, kind="ExternalInput")
    rowsd = nc.dram_tensor("rowsd", [1, 8 * C], F32, kind="ExternalInput")

    out_sl = nc.dram_tensor("out_sl", [B, C, LC], F32, kind="ExternalOutput")

    cst_np, sel_np, bh_np = _consts()
    c_cst = nc.inline_tensor(cst_np, name="c_cst")
    c_sel = nc.inline_tensor(sel_np, name="c_sel")
    c_bh = nc.inline_tensor(bh_np, name="c_bh")
    c_onesk = nc.inline_tensor(np.full((128, 1), 1.0 / C, np.float16),
                               name="c_onesk")
    c_ones = nc.inline_tensor(np.ones((1, B * LC), np.float32), name="c_ones")

    NCST = cst_np.shape[1]
    O_DCT, O_XN, O_BLK, O_RV = 0, KDEG, KDEG + 1, KDEG + 1 + 128

    dbg = {}
    if DBG:
        for nm, shp in [("d_tts", [H, T]), ("d_tt", [128, FP]),
                        ("d_cb", [128, KDEG]), ("d_wp", [128, FP]),
                        ("d_wht", [H, T]), ("d_y", [128, KC * T]),
                        ("d_x", [128, KC * T]), ("d_f", [128, 1])]:
            dbg[nm] = nc.dram_tensor(nm, shp, F32, kind="ExternalOutput")

    with tile.TileContext(nc) as tc, ExitStack() as ctx:
        p_big = ctx.enter_context(tc.tile_pool(name="big", bufs=1))
        p_act = ctx.enter_context(tc.tile_pool(name="act", bufs=4))
        p_sm = ctx.enter_context(tc.tile_pool(name="sm", bufs=1))
        p_cl = ctx.enter_context(tc.tile_pool(name="cl", bufs=1))
        ps_mm = ctx.enter_context(tc.tile_pool(name="psmm", bufs=4, space="PSUM"))
        ps_x = ctx.enter_context(tc.tile_pool(name="psx", bufs=4, space="PSUM"))

        # ---- critical loads (sync queue): xs then expv -------------------
        xs = p_big.tile([128, KC * T], F32R, tag="xs")
        nc.sync.dma_start(xs[:], seq_sl[:].bitcast(F32R))
        e2 = p_sm.tile([2, G], F32R, tag="e2")
        nc.sync.dma_start(e2[:], expv[:].bitcast(F32R))

        # ---- packed smalls (gpsimd queue; keep ACT queue compute-only) ---
        m_sb = p_sm.tile([128, KC * H], F32R, tag="msb")
        nc.gpsimd.dma_start(m_sb[:], md[:].bitcast(F32R))
        v_sb = p_sm.tile([128, KC * H], F32R, tag="vsb")
        nc.gpsimd.dma_start(v_sb[:], vd[:].bitcast(F32R))
        cst_sb = p_sm.tile([128, NCST], F32R, tag="cst")
        nc.gpsimd.dma_start(cst_sb[:], c_cst[:].bitcast(F32R))
        pc = p_sm.tile([128, KC * NPC], F32R, tag="pc")
        nc.gpsimd.dma_start(pc[:], pcd[:].bitcast(F32R))
        rows_sb = p_sm.tile([1, 8 * C], F32R, tag="rows")
        nc.gpsimd.dma_start(rows_sb[:], rowsd[:].bitcast(F32R))
        sel_sb = p_sm.tile([H, C + 512], F32R, tag="sel")
        nc.gpsimd.dma_start(sel_sb[:], c_sel[:].bitcast(F32R))
        bh_sb = p_sm.tile([2, 128], F32R, tag="bh")
        nc.gpsimd.dma_start(bh_sb[:], c_bh[:].bitcast(F32R))
        b1c = p_sm.tile([128, KH], F32, tag="b1c")
        nc.gpsimd.dma_start(b1c[:], b1d[:])
        onesk_h = p_sm.tile([128, 1], FP16, tag="onesk")
        nc.gpsimd.dma_start(onesk_h[:], c_onesk[:])
        ones_row = p_sm.tile([1, T], F32R, tag="ones")
        nc.gpsimd.dma_start(ones_row[:], c_ones[:].bitcast(F32R))

        # ---- bulk fp16 weights on the gpsimd queue -----------------------
        w1_sb = p_big.tile([128, KC * 4 * C], FP16, tag="w1")
        nc.gpsimd.dma_start(w1_sb[:], w1t[:])
        w2_sb = p_big.tile([128, KH * C], FP16, tag="w2")
        nc.gpsimd.dma_start(w2_sb[:], w2t[:])
        wo_sb = p_big.tile([128, KC * C], FP16, tag="wo")
        nc.gpsimd.dma_start(wo_sb[:], wot[:])

        def pccol_r(kt, j, n=1):
            return pc[:, kt * NPC + j:kt * NPC + j + n]

        def pccol(kt, j, n=1):
            return pccol_r(kt, j, n).bitcast(F32)

        eps_col = p_sm.tile([1, 1], F32, tag="epsc")
        nc.vector.memset(eps_col[:], EPS)

        # ---- a path: tt = a/SCAL in [H, T]; clamp; repack to [128, 32] ---
        pa = ps_mm.tile([H, T], F32, tag="mm", name="pa")
        for kt in range(KC):
            nc.tensor.matmul(pa[:], m_sb[:, kt * H:(kt + 1) * H],
                             xs[:, kt * T:(kt + 1) * T],
                             start=(kt == 0), stop=(kt == KC - 1))
        tt_sb = p_sm.tile([H, T], F32, tag="tts")
        nc.scalar.activation(tt_sb[:], pa[:], AF.Identity,
                             bias=pccol(0, 5)[0:H, :], scale=SCALE / SCAL)
        nc.vector.tensor_scalar_max(tt_sb[:], tt_sb[:], -1.0)
        tts_r = p_sm.tile([H, T], F32R, tag="ttsr")
        nc.vector.tensor_scalar_min(tts_r[:], tt_sb[:], 1.0)
        ptt = ps_mm.tile([128, FP], F32, tag="mm", name="ptt")
        for b in range(B):
            for lhi in range(8):
                nc.tensor.matmul(
                    ptt[b * 64:(b + 1) * 64, :],
                    sel_sb[:, C + lhi * 64:C + (lhi + 1) * 64].bitcast(F32),
                    tts_r[:, b * LC + lhi * FP:b * LC + (lhi + 1) * FP]
                    .bitcast(F32),
                    start=(lhi == 0), stop=(lhi == 7))
        tt = p_cl.tile([128, FP], F32, tag="tt")
        nc.vector.tensor_copy(tt[:], ptt[:])

        # ---- both-batch softmax collapse at 64 Chebyshev nodes -----------
        # e chunk broadcast via K=2 matmul; exp + weighted sums per chunk
        pn = p_cl.tile([128, G], F32, tag="ndB")
        zc = p_sm.tile([128, GC], F32, tag="zc")
        nmc = p_sm.tile([128, GC], F32, tag="nmc")
        for i in range(GC):
            sl = slice(i * 512, (i + 1) * 512)
            ep = ps_mm.tile([128, 512], F32, tag="mm", name=f"ep{i}")
            nc.tensor.matmul(ep[:], bh_sb[:], e2[:, sl], start=True, stop=True)
            nc.scalar.activation(pn[:, sl], ep[:], AF.Exp,
                                 scale=cst_sb[:, O_XN:O_XN + 1].bitcast(F32),
                                 accum_out=zc[:, i:i + 1])
            nc.vector.scalar_tensor_tensor(
                out=pn[:, sl], in0=pn[:, sl], scalar=1.0, in1=ep[:],
                op0=OP.mult, op1=OP.mult, accum_out=nmc[:, i:i + 1])
        z_col = p_sm.tile([128, 1], F32, tag="zcol")
        nc.vector.tensor_reduce(z_col[:], zc[:], axis=AX.X, op=OP.add)
        nm_col = p_sm.tile([128, 1], F32, tag="nmcol")
        nc.vector.tensor_reduce(nm_col[:], nmc[:], axis=AX.X, op=OP.add)
        zr_col = p_sm.tile([128, 1], F32, tag="zrc")
        nc.vector.reciprocal(zr_col[:], z_col[:])
        f_col = p_sm.tile([128, 1], F32, tag="fc")
        nc.vector.tensor_mul(f_col[:], nm_col[:], zr_col[:])
        if DBG:
            nc.sync.dma_start(dbg["d_f"][:], f_col[:])

        # ---- early LN1 stats from xs (y = xs + uv*w_h + cv is low-rank) --
        def pcell(kt):
            return pc[0:1, kt * NPC + 6:kt * NPC + 7]

        st0 = ps_x.tile([1, T], F32, tag="x", name="st0a")
        st1a = ps_x.tile([1, T], F32, tag="x", name="st1a")
        st1b = ps_x.tile([1, T], F32, tag="x", name="st1b")
        qu = ps_x.tile([H, T], F32, tag="x", name="qu")
        sqx_t = []
        for kt in range(KC):
            sqx = p_act.tile([128, T], FP16, tag="sqx", bufs=2, name=f"sqx{kt}")
            nc.scalar.activation(sqx[:], xs[:, kt * T:(kt + 1) * T].bitcast(F32),
                                 AF.Square)
            sqx_t.append(sqx)
        for kt in range(KC):
            nc.tensor.matmul(st1a[:], onesk_h[:], sqx_t[kt][:],
                             start=(kt == 0), stop=(kt == KC - 1))
        for kt in range(KC):
            nc.tensor.matmul(st0[:], pccol_r(kt, 3), xs[:, kt * T:(kt + 1) * T],
                             start=(kt == 0), stop=False)
        for kt in range(KC):
            nc.tensor.matmul(st1b[:], pccol_r(kt, 4), xs[:, kt * T:(kt + 1) * T],
                             start=(kt == 0), stop=False)
        nc.tensor.matmul(st1b[:], pcell(2), ones_row[:], start=False, stop=False)
        for kt in range(KC):
            nc.tensor.matmul(qu[:], v_sb[:, kt * H:(kt + 1) * H],
                             xs[:, kt * T:(kt + 1) * T],
                             start=(kt == 0), stop=(kt == KC - 1))

        # cb[p, k] = ck[batch(p), k] via block-ones matmul (no DRAM trip)
        fdct = p_sm.tile([128, KDEG], F32R, tag="fdct")
        nc.vector.tensor_scalar_mul(
            fdct[:], cst_sb[:, O_DCT:O_DCT + KDEG].bitcast(F32), f_col[:])
        pcb = ps_mm.tile([128, KDEG], F32, tag="mm", name="pcb")
        nc.tensor.matmul(pcb[:], cst_sb[:, O_BLK:O_BLK + 128], fdct[:],
                         start=True, stop=True)
        cb = p_cl.tile([128, KDEG], F32, tag="cb")
        nc.vector.tensor_copy(cb[:], pcb[:])
        if DBG:
            nc.sync.dma_start(dbg["d_tts"][:], tt_sb[:])
            nc.sync.dma_start(dbg["d_tt"][:], tt[:])
            nc.sync.dma_start(dbg["d_cb"][:], cb[:])

        # ---- Clenshaw over packed a: [128, 32] ---------------------------
        tt2 = p_cl.tile([128, FP], F32, tag="tt2")
        nc.vector.tensor_add(tt2[:], tt[:], tt[:])
        bb1 = p_cl.tile([128, FP], F32, tag="bb1")
        bb2 = p_cl.tile([128, FP], F32, tag="bb2")
        tmp = p_cl.tile([128, FP], F32, tag="tmp")
        nc.vector.memset(bb1[:], 0.0)
        nc.vector.memset(bb2[:], 0.0)
        cur1, cur2 = bb1, bb2
        for k in range(KDEG - 1, 0, -1):
            nc.vector.tensor_mul(tmp[:], tt2[:], cur1[:])
            nc.vector.scalar_tensor_tensor(
                out=cur2[:], in0=tmp[:], scalar=cb[:, k:k + 1], in1=cur2[:],
                op0=OP.add, op1=OP.subtract)
            cur1, cur2 = cur2, cur1
        w_pack = p_cl.tile([128, FP], F32R, tag="wp")
        nc.vector.tensor_mul(tmp[:], tt[:], cur1[:])
        nc.vector.scalar_tensor_tensor(
            out=w_pack[:], in0=tmp[:], scalar=cb[:, 0:1], in1=cur2[:],
            op0=OP.add, op1=OP.subtract)
        if DBG:
            nc.sync.dma_start(dbg["d_wp"][:], w_pack[:].bitcast(F32))

        # ---- w to [H, T] layout via selector matmuls ---------------------
        pw = ps_mm.tile([H, T], F32, tag="mm", name="pw")
        for b in range(B):
            for lhi in range(8):
                jb = b * 8 + lhi
                nc.tensor.matmul(
                    pw[:, b * LC + lhi * FP:b * LC + (lhi + 1) * FP],
                    cst_sb[:, O_RV + jb * H:O_RV + (jb + 1) * H],
                    w_pack[:], start=True, stop=True)
        w_HT = p_sm.tile([H, T], F32R, tag="wht")
        nc.vector.tensor_copy(w_HT[:], pw[:])
        if DBG:
            nc.sync.dma_start(dbg["d_wht"][:], w_HT[:].bitcast(F32))

        # ---- finish LN1 stats with the w-dependent low-rank terms --------
        # st0 += (su'/C) @ w_HT + (sum cv)/C * ones
        nc.tensor.matmul(st0[:], pccol_r(1, 5)[0:H, :], w_HT[:],
                         start=False, stop=False)
        nc.tensor.matmul(st0[:], pcell(1), ones_row[:], start=False, stop=True)
        wqu = p_sm.tile([H, T], F32R, tag="wqu")
        nc.vector.tensor_mul(wqu[:], w_HT[:].bitcast(F32), qu[:])
        wsq = p_sm.tile([H, T], F32R, tag="wsq")
        nc.vector.tensor_mul(wsq[:], w_HT[:].bitcast(F32), w_HT[:].bitcast(F32))
        nc.tensor.matmul(st1b[:], pccol_r(2, 5)[0:H, :], wqu[:],
                         start=False, stop=False)
        nc.tensor.matmul(st1b[:], pccol_r(3, 5)[0:H, :], wsq[:],
                         start=False, stop=False)
        nc.tensor.matmul(st1b[:], pccol_r(0, 6)[0:H, :], w_HT[:],
                         start=False, stop=True)

        # ---- x_attn + residual -> y via selector matmul ------------------
        y_t = []
        for mt in range(KC):
            wr = ps_mm.tile([128, T], F32, tag="mm", name=f"wr{mt}")
            nc.tensor.matmul(wr[:], sel_sb[:, mt * 128:(mt + 1) * 128],
                             w_HT[:], start=True, stop=False)
            nc.tensor.matmul(wr[:], rows_sb[0:1, mt * 128:(mt + 1) * 128],
                             ones_row[:], start=False, stop=True)
            yk = p_act.tile([128, T], FP16, tag="y", name=f"y{mt}")
            # yk = (wr * uv) + xs   (wr already contains w_bcast + cv)
            nc.vector.scalar_tensor_tensor(
                out=yk[:], in0=wr[:], scalar=pccol(mt, 1),
                in1=xs[:, mt * T:(mt + 1) * T].bitcast(F32),
                op0=OP.mult, op1=OP.add)
            y_t.append(yk)
        if DBG:
            for mt in range(KC):
                nc.gpsimd.dma_start(dbg["d_y"][:, mt * T:(mt + 1) * T],
                                    y_t[mt][:])

        def ln_rows(s0, s1a, s1b, ph):
            """stat psums -> (rstd_row F32R psum-broadcastable, q_row F32R)."""
            musq = p_sm.tile([1, T], F32, tag="lnrow", bufs=4, name=f"musq{ph}")
            nc.scalar.activation(musq[:], s0[:], AF.Square)
            var = p_sm.tile([1, T], F32, tag="lnrow", bufs=4, name=f"var{ph}")
            if s1b is not None:
                t1 = p_sm.tile([1, T], F32, tag="lnrow", bufs=4, name=f"t1{ph}")
                nc.vector.tensor_sub(t1[:], s1a[:], musq[:])
                nc.vector.tensor_add(var[:], t1[:], s1b[:])
            else:
                nc.vector.tensor_sub(var[:], s1a[:], musq[:])
            rstd_f32 = p_sm.tile([1, T], F32, tag="lnrow", bufs=4, name=f"rsf{ph}")
            nc.scalar.activation(rstd_f32[:], var[:], AF.Abs_reciprocal_sqrt,
                                 bias=eps_col[:])
            rstd_row = p_sm.tile([1, T], F32R, tag="lnrow", bufs=4,
                                 name=f"rstd{ph}")
            nc.vector.tensor_copy(rstd_row[:], rstd_f32[:])
            q_row = p_sm.tile([1, T], F32R, tag="lnrow", bufs=4, name=f"q{ph}")
            nc.vector.tensor_mul(q_row[:], s0[:], rstd_f32[:])
            return rstd_row, q_row

        def xt_tiles(y_tiles, rstd_row, ph):
            """x~ = y * rstd (per token) as fp16 tiles."""
            outs = []
            for kt in range(KC):
                pR = ps_mm.tile([128, T], F32, tag="mm", name=f"pR{ph}{kt}")
                nc.tensor.matmul(pR[:], ones_row[0:1, 0:128], rstd_row[:],
                                 start=True, stop=True)
                xo = p_act.tile([128, T], FP16, tag=f"ln{ph}", bufs=4,
                                name=f"ln{ph}{kt}")
                nc.vector.tensor_mul(xo[:], y_tiles[kt][:], pR[:])
                outs.append(xo)
            return outs

        rstd1, q1 = ln_rows(st0, st1a, st1b, "a")
        x_t = xt_tiles(y_t, rstd1, "a")
        if DBG:
            for mt in range(KC):
                nc.gpsimd.dma_start(dbg["d_x"][:, mt * T:(mt + 1) * T],
                                    x_t[mt][:])

        # ---- FFN1: h = relu(W1g @ x~ + r1*q1 + b1') ----------------------
        h_t = []
        for mt in range(KH):
            pf = ps_mm.tile([128, T], F32, tag="mm", name=f"pf1{mt}")
            for kt in range(KC):
                sl = slice(kt * 4 * C + mt * 128, kt * 4 * C + (mt + 1) * 128)
                nc.tensor.matmul(pf[:], w1_sb[:, sl], x_t[kt][:],
                                 start=(kt == 0), stop=False)
            nc.tensor.matmul(pf[:], rows_sb[0:1, 4 * C + mt * 128:
                                            4 * C + (mt + 1) * 128],
                             q1[:], start=False, stop=True)
            hm = p_big.tile([128, T], FP16, tag="h", bufs=16, name=f"h{mt}")
            nc.scalar.activation(hm[:], pf[:], AF.Relu, bias=b1c[:, mt:mt + 1])
            h_t.append(hm)

        # ---- FFN2 + residual: y2 = g1*x~ - g1*q1 + be1 + W2@h + b2 -------
        y2_t = []
        for mt in range(KC):
            pf = ps_mm.tile([128, T], F32, tag="mm", name=f"pf2{mt}")
            for kt in range(KH):
                sl = slice(kt * C + mt * 128, kt * C + (mt + 1) * 128)
                nc.tensor.matmul(pf[:], w2_sb[:, sl], h_t[kt][:],
                                 start=(kt == 0), stop=False)
            nc.tensor.matmul(pf[:], rows_sb[0:1, C + mt * 128:C + (mt + 1) * 128],
                             q1[:], start=False, stop=False)
            nc.tensor.matmul(pf[:],
                             rows_sb[0:1, 2 * C + mt * 128:2 * C + (mt + 1) * 128],
                             ones_row[:], start=False, stop=True)
            y2 = p_act.tile([128, T], FP16, tag="y", name=f"y2{mt}")
            nc.vector.scalar_tensor_tensor(
                out=y2[:], in0=x_t[mt][:], scalar=pccol(mt, 2),
                in1=pf[:], op0=OP.mult, op1=OP.add)
            y2_t.append(y2)

        # ---- LN2 stats (classic) -----------------------------------------
        st0b = ps_x.tile([1, T], F32, tag="x", name="st0b")
        st1c = ps_x.tile([1, T], F32, tag="x", name="st1c")
        for kt in range(KC):
            nc.tensor.matmul(st0b[:], onesk_h[:], y2_t[kt][:],
                             start=(kt == 0), stop=(kt == KC - 1))
        sq_t = []
        for kt in range(KC):
            sq = p_act.tile([128, T], FP16, tag="sqx", bufs=2, name=f"sqb{kt}")
            nc.scalar.activation(sq[:], y2_t[kt][:], AF.Square)
            sq_t.append(sq)
        for kt in range(KC):
            nc.tensor.matmul(st1c[:], onesk_h[:], sq_t[kt][:],
                             start=(kt == 0), stop=(kt == KC - 1))
        rstd2, q2 = ln_rows(st0b, st1c, None, "b")
        z_t = xt_tiles(y2_t, rstd2, "b")

        # ---- output: out = Wog2 @ z~ + r3*q2 + bo' -----------------------
        for mt in range(KC):
            pf = ps_mm.tile([128, T], F32, tag="mm", name=f"pfo{mt}")
            for kt in range(KC):
                sl = slice(kt * C + mt * 128, kt * C + (mt + 1) * 128)
                nc.tensor.matmul(pf[:], wo_sb[:, sl], z_t[kt][:],
                                 start=(kt == 0), stop=False)
            nc.tensor.matmul(pf[:],
                             rows_sb[0:1, 3 * C + mt * 128:3 * C + (mt + 1) * 128],
                             q2[:], start=False, stop=True)
            om = p_act.tile([128, T], F32, tag="tmpx", bufs=2, name=f"om{mt}")
            nc.scalar.activation(om[:], pf[:], AF.Identity, bias=pccol(mt, 0))
            nc.sync.dma_start(
                out_sl[:, mt * 128:(mt + 1) * 128, :].rearrange("b c l -> c b l"),
                om[:])

    nc.compile()
    return nc


def kernel(**inputs):
    global _CACHE, LAST_RESULTS
    if _CACHE is None:
        _CACHE = _build()
    nc = _CACHE

    f32 = lambda x: np.asarray(x, dtype=np.float32)
    f16t = lambda x: np.ascontiguousarray(np.asarray(x).T, dtype=np.float16)
    seq = f32(inputs["seq"])

    # host-side stage A: all weight-only precomputation (exact fp32 math)
    Wg = f32(inputs["Wg"])[:, 0]
    bg = f32(inputs["bg"])
    Wk, Wv, Wq = f32(inputs["Wk"]), f32(inputs["Wv"]), f32(inputs["Wq"])
    bq, bv = f32(inputs["bq"]), f32(inputs["bv"])
    g1, be1 = f32(inputs["g1"]), f32(inputs["beta1"])
    g2, be2 = f32(inputs["g2"]), f32(inputs["beta2"])
    W1, b1 = f32(inputs["W1"]), f32(inputs["b1"])
    W2, b2 = f32(inputs["W2"]), f32(inputs["b2"])
    Wo, bo = f32(inputs["Wo"]), f32(inputs["bo"])
    uk = Wk @ Wg
    uv = Wv @ Wg
    cv = Wv @ bg + bv
    mask = np.zeros((C, H), np.float32)
    for h in range(H):
        mask[h * D:(h + 1) * D, h] = 1.0
    U = mask * uk[:, None]
    V = mask * uv[:, None]
    M = (Wq.T @ U).astype(np.float32)                      # [C, H]
    a0s = ((U.T @ bq) * SCALE / SCAL).astype(np.float32)   # [H]

    # LN folds
    W1g = W1 * g1[None, :]
    Wog2 = Wo * g2[None, :]
    b1p = b1 + W1 @ be1
    bop = bo + Wo @ be2
    r1 = -W1g.sum(axis=1)                                  # [4C]
    r3 = -Wog2.sum(axis=1)                                 # [C]

    # per-kt packed columns
    miscA = np.zeros((KC, 128), np.float32)
    miscA[0, :H] = a0s
    miscA[1, :H] = (V.sum(axis=0)) / C                     # su'/C
    miscA[2, :H] = 2.0 / C
    miscA[3, :H] = ((uv * uv)[:, None] * mask).sum(axis=0) / C
    miscB = np.zeros((KC, 128), np.float32)
    miscB[0, :H] = 2.0 * ((uv * cv)[:, None] * mask).sum(axis=0) / C
    miscB[1, 0] = cv.sum() / C
    miscB[2, 0] = (cv * cv).sum() / C
    cols = [bop.reshape(KC, 128), uv.reshape(KC, 128), g1.reshape(KC, 128),
            np.full((KC, 128), 1.0 / C, np.float32),
            (2.0 * cv / C).reshape(KC, 128), miscA, miscB]
    pcd = np.ascontiguousarray(
        np.stack(cols, axis=2).transpose(1, 0, 2).reshape(128, KC * NPC))
    md = np.ascontiguousarray(
        M.reshape(KC, 128, H).transpose(1, 0, 2).reshape(128, KC * H))
    vd_img = np.ascontiguousarray(
        V.reshape(KC, 128, H).transpose(1, 0, 2).reshape(128, KC * H))
    b1d = np.ascontiguousarray(b1p.reshape(KH, 128).T)
    rowsd = np.ascontiguousarray(np.concatenate(
        [cv, -g1, be1 + b2, r3, r1]).reshape(1, 8 * C))

    def wimg(wT_f16, ksplit):
        # [Cin, Cout] -> [128, ksplit*Cout] image (partition-folded)
        cin, cout = wT_f16.shape
        return np.ascontiguousarray(
            wT_f16.reshape(ksplit, 128, cout).transpose(1, 0, 2).reshape(
                128, ksplit * cout))

    f16 = lambda x: np.ascontiguousarray(x.T, dtype=np.float16)
    base = {
        "expv": f32(inputs["exp"]),
        "w1t": wimg(f16(W1g), KC),
        "w2t": wimg(f16t(inputs["W2"]), KH),
        "wot": wimg(f16(Wog2), KC),
        "pcd": pcd,
        "md": md,
        "vd": vd_img,
        "b1d": b1d,
        "rowsd": rowsd,
    }
    # xs image per core: [128, kt*T + b*LC + l] = seq[b, kt*128+p, c0+l]
    seq_r = seq.reshape(B, KC, 128, L)
    in_maps = []
    for c in range(NCORES):
        m = dict(base)
        sl = seq_r[:, :, :, c * LC:(c + 1) * LC]           # [B, KC, 128, LC]
        m["seq_sl"] = np.ascontiguousarray(
            sl.transpose(2, 1, 0, 3).reshape(128, KC * T))
        in_maps.append(m)

    res = run_bass_kernel_spmd(nc, in_maps, list(range(NCORES)), trace=TRACE,
                               **TRACE_KW)
    LAST_RESULTS = res
    out = np.empty((B, C, L), np.float32)
    for c in range(NCORES):
        out[:, :, c * LC:(c + 1) * LC] = res.results[c]["out_sl"]
    return out


# revision 31
# speedup vs baseline: 1.0205x; 1.0205x over previous
"""Trainium2 Bass kernel for nn_G3DCrossAttention (B=2, C=512, L=2048, G=2048, H=8).

Exact-math rank-1 collapse of the attention (see kernel_v1_baseline.py for the
derivation): exp_p is rank-1 in channels, so per head the attention output is
x_attn = w*u_v + c_v with w = f_b(a), a = x_seq @ M + a0. f_b is evaluated at
64 Chebyshev nodes on device (exact softmax-collapse over all G genes), fit
with a KDEG-term Chebyshev series and evaluated by a Clenshaw recurrence.

v3 structure (vs the 175us baseline):
  - u_k/u_v/c_v/M/a0 depend only on weights -> precomputed host-side in numpy
    and shipped as packed constants (kills the 38us on-device stage A and 3MB
    of Wq/Wk/Wv DMA traffic)
  - e_b node matrix built by a K=2 block-ones matmul from a [2,G] tile instead
    of a broadcast DMA (whose descriptor generation took 21us to issue)
  - Chebyshev coefficients broadcast by a block-ones matmul (no DRAM trip)
  - per-head w broadcast by a selector matmul from a [H,T] tile; c_v folded in
    as a K=1 matmul; the [H,T] tile comes from a 2-DMA SBUF->SBUF repack
  - LN rstd via ACT Abs_reciprocal_sqrt (40000-bucket table; one table switch
    total) instead of single-lane reciprocal (3.3us) or Ln+Exp (table thrash)
  - all constants packed into a handful of DMAs; fp16 weights one DMA each
  - KDEG=16 (w err ~4e-4; full-pipeline fp32 err 2.5e-4; gate is 2e-2)

Sharding: data-parallel over L across 8 cores (L/8 = 256 queries each).
"""

from contextlib import ExitStack

import numpy as np

import concourse.bass as bass
import concourse.tile as tile
from concourse import bacc, mybir
from concourse.bass_utils import run_bass_kernel_spmd

F32 = mybir.dt.float32
F32R = mybir.dt.float32r
FP16 = mybir.dt.float16
AF = mybir.ActivationFunctionType
OP = mybir.AluOpType
AX = mybir.AxisListType

B, C, L, G, H = 2, 512, 2048, 2048, 8
D = C // H
NCORES = 8
LC = L // NCORES              # 256 queries per core
T = B * LC                    # 512 tokens per core (tau = b*LC + l)
KC = C // 128                 # 4 partition tiles over C
KH = (4 * C) // 128           # 16 partition tiles over 4C
FP = LC // 8                  # 32: free dim of the packed a/w tiles
GC = G // 512                 # 4 chunks over genes
SCALE = 1.0 / float(np.sqrt(D))
EPS = 1e-5
SCAL = 5.0                    # Chebyshev half-range in a-units (|a|max ~ 4.43)
KDEG = 12                     # Chebyshev series length
MNODES = 64                   # Chebyshev nodes per batch (2 batches -> 128 parts)
NPC = 7                       # per-kt cols: boP uv g1 onesC cv2C miscA miscB

TRACE = False
TRACE_KW = {}
LAST_RESULTS = None
DBG = False

_CACHE = None


def _consts():
    m = np.arange(MNODES)
    theta = np.pi * (2 * m + 1) / (2 * MNODES)
    xn64 = (SCAL * np.cos(theta)).astype(np.float32)
    xnodes = np.concatenate([xn64, xn64])                 # [128] both batches
    dct1 = np.zeros((MNODES, KDEG), np.float32)
    for k in range(KDEG):
        dct1[:, k] = (2.0 / MNODES) * np.cos(k * theta)
    dct1[:, 0] *= 0.5
    dct_full = np.concatenate([dct1, dct1], axis=0)       # [128, KDEG]
    blockones = np.zeros((128, 128), np.float32)
    blockones[:64, :64] = 1.0
    blockones[64:, 64:] = 1.0
    # reverse repack selector: w_pack [128,FP] -> w_HT [8,T]
    # c_rv[q, (b*8+lhi)*8 + h] = 1 iff q == b*64 + h*8 + lhi
    rv = np.zeros((128, 128), np.float32)
    for b in range(2):
        for lhi in range(8):
            for h in range(H):
                rv[b * 64 + h * 8 + lhi, (b * 8 + lhi) * 8 + h] = 1.0
    # cst layout: [dct KDEG][xn 1][blockones 128][rv 128]
    cst = np.concatenate([dct_full, xnodes[:, None], blockones, rv], axis=1)
    sel = np.zeros((H, C), np.float32)                    # sel[h, c] = [c//D == h]
    for h in range(H):
        sel[h, h * D:(h + 1) * D] = 1.0
    # forward repack selector: tt_sb [8,T] -> tt [128,FP]
    # rp[j, lhi*64 + p] = 1 iff p == j*8 + lhi   (per batch-half)
    rp = np.zeros((H, 512), np.float32)
    for lhi in range(8):
        for j in range(H):
            rp[j, lhi * 64 + j * 8 + lhi] = 1.0
    sel2 = np.concatenate([sel, rp], axis=1)              # [8, C + 512]
    bh = np.zeros((2, 128), np.float32)                   # batch-half selector
    bh[0, :64] = 1.0
    bh[1, 64:] = 1.0
    return cst, sel2, bh


def _build():
    nc = bacc.Bacc(debug=False, num_devices=NCORES)

    # ---- external inputs -------------------------------------------------
    seq_sl = nc.dram_tensor("seq_sl", [128, KC * T], F32, kind="ExternalInput")
    expv = nc.dram_tensor("expv", [B, G], F32, kind="ExternalInput")
    w1t = nc.dram_tensor("w1t", [128, KC * 4 * C], FP16, kind="ExternalInput")
    w2t = nc.dram_tensor("w2t", [128, KH * C], FP16, kind="ExternalInput")
    wot = nc.dram_tensor("wot", [128, KC * C], FP16, kind="ExternalInput")
    pcd = nc.dram_tensor("pcd", [128, KC * NPC], F32, kind="ExternalInput")
    md = nc.dram_tensor("md", [128, KC * H], F32, kind="ExternalInput")  # M tiles
    vd = nc.dram_tensor("vd", [128, KC * H], F32, kind="ExternalInput")  # V tiles
    b1d = nc.dram_tensor("b1d", [128, KH], F32, kind="ExternalInput")
    rowsd = nc.dram_tensor("rowsd", [1, 8 * C], F32, kind="ExternalInput")

    out_sl = nc.dram_tensor("out_sl", [B, C, LC], F32, kind="ExternalOutput")
    warm_d = nc.dram_tensor("warm_d", [2, FP], F32, kind="ExternalOutput")

    cst_np, sel_np, bh_np = _consts()
    c_cst = nc.inline_tensor(cst_np, name="c_cst")
    c_sel = nc.inline_tensor(sel_np, name="c_sel")
    c_bh = nc.inline_tensor(bh_np, name="c_bh")
    c_onesk = nc.inline_tensor(np.full((128, 1), 1.0 / C, np.float16),
                               name="c_onesk")
    c_ones = nc.inline_tensor(np.ones((1, B * LC), np.float32), name="c_ones")

    NCST = cst_np.shape[1]
    O_DCT, O_XN, O_BLK, O_RV = 0, KDEG, KDEG + 1, KDEG + 1 + 128

    dbg = {}
    if DBG:
        for nm, shp in [("d_tts", [H, T]), ("d_tt", [128, FP]),
                        ("d_cb", [128, KDEG]), ("d_wp", [128, FP]),
                        ("d_wht", [H, T]), ("d_y", [128, KC * T]),
                        ("d_x", [128, KC * T]), ("d_f", [128, 1])]:
            dbg[nm] = nc.dram_tensor(nm, shp, F32, kind="ExternalOutput")

    with tile.TileContext(nc) as tc, ExitStack() as ctx:
        p_big = ctx.enter_context(tc.tile_pool(name="big", bufs=1))
        p_act = ctx.enter_context(tc.tile_pool(name="act", bufs=4))
        p_sm = ctx.enter_context(tc.tile_pool(name="sm", bufs=1))
        p_cl = ctx.enter_context(tc.tile_pool(name="cl", bufs=1))
        ps_mm = ctx.enter_context(tc.tile_pool(name="psmm", bufs=4, space="PSUM"))
        ps_x = ctx.enter_context(tc.tile_pool(name="psx", bufs=4, space="PSUM"))

        # ---- critical loads (sync queue): xs then expv -------------------
        xs = p_big.tile([128, KC * T], F32R, tag="xs")
        nc.sync.dma_start(xs[:], seq_sl[:].bitcast(F32R))
        e2 = p_sm.tile([2, G], F32R, tag="e2")
        nc.sync.dma_start(e2[:], expv[:].bitcast(F32R))

        # ---- packed smalls (gpsimd queue; keep ACT queue compute-only) ---
        m_sb = p_sm.tile([128, KC * H], F32R, tag="msb")
        nc.gpsimd.dma_start(m_sb[:], md[:].bitcast(F32R))
        v_sb = p_sm.tile([128, KC * H], F32R, tag="vsb")
        nc.gpsimd.dma_start(v_sb[:], vd[:].bitcast(F32R))
        cst_sb = p_sm.tile([128, NCST], F32R, tag="cst")
        nc.gpsimd.dma_start(cst_sb[:], c_cst[:].bitcast(F32R))
        pc = p_sm.tile([128, KC * NPC], F32R, tag="pc")
        nc.gpsimd.dma_start(pc[:], pcd[:].bitcast(F32R))
        rows_sb = p_sm.tile([1, 8 * C], F32R, tag="rows")
        nc.gpsimd.dma_start(rows_sb[:], rowsd[:].bitcast(F32R))
        sel_sb = p_sm.tile([H, C + 512], F32R, tag="sel")
        nc.gpsimd.dma_start(sel_sb[:], c_sel[:].bitcast(F32R))
        bh_sb = p_sm.tile([2, 128], F32R, tag="bh")
        nc.gpsimd.dma_start(bh_sb[:], c_bh[:].bitcast(F32R))
        b1c = p_sm.tile([128, KH], F32, tag="b1c")
        nc.gpsimd.dma_start(b1c[:], b1d[:])
        onesk_h = p_sm.tile([128, 1], FP16, tag="onesk")
        nc.gpsimd.dma_start(onesk_h[:], c_onesk[:])
        ones_row = p_sm.tile([1, T], F32R, tag="ones")
        nc.gpsimd.dma_start(ones_row[:], c_ones[:].bitcast(F32R))

        # ---- bulk fp16 weights on the gpsimd queue -----------------------
        w1_sb = p_big.tile([128, KC * 4 * C], FP16, tag="w1")
        nc.gpsimd.dma_start(w1_sb[:], w1t[:])
        w2_sb = p_big.tile([128, KH * C], FP16, tag="w2")
        nc.gpsimd.dma_start(w2_sb[:], w2t[:])
        wo_sb = p_big.tile([128, KC * C], FP16, tag="wo")
        nc.gpsimd.dma_start(wo_sb[:], wot[:])

        def pccol_r(kt, j, n=1):
            return pc[:, kt * NPC + j:kt * NPC + j + n]

        def pccol(kt, j, n=1):
            return pccol_r(kt, j, n).bitcast(F32)

        eps_col = p_sm.tile([1, 1], F32, tag="epsc")
        nc.vector.memset(eps_col[:], EPS)

        # ---- a path: tt = a/SCAL in [H, T]; clamp; repack to [128, 32] ---
        pa = ps_mm.tile([H, T], F32, tag="mm", name="pa")
        for kt in range(KC):
            nc.tensor.matmul(pa[:], m_sb[:, kt * H:(kt + 1) * H],
                             xs[:, kt * T:(kt + 1) * T],
                             start=(kt == 0), stop=(kt == KC - 1))
        tt_sb = p_sm.tile([H, T], F32, tag="tts")
        nc.scalar.activation(tt_sb[:], pa[:], AF.Identity,
                             bias=pccol(0, 5)[0:H, :], scale=SCALE / SCAL)
        nc.vector.tensor_scalar_max(tt_sb[:], tt_sb[:], -1.0)
        tts_r = p_sm.tile([H, T], F32R, tag="ttsr")
        nc.vector.tensor_scalar_min(tts_r[:], tt_sb[:], 1.0)
        ptt = ps_mm.tile([128, FP], F32, tag="mm", name="ptt")
        for b in range(B):
            for lhi in range(8):
                nc.tensor.matmul(
                    ptt[b * 64:(b + 1) * 64, :],
                    sel_sb[:, C + lhi * 64:C + (lhi + 1) * 64].bitcast(F32),
                    tts_r[:, b * LC + lhi * FP:b * LC + (lhi + 1) * FP]
                    .bitcast(F32),
                    start=(lhi == 0), stop=(lhi == 7))
        tt = p_cl.tile([128, FP], F32, tag="tt")
        nc.vector.tensor_copy(tt[:], ptt[:])

        # ---- both-batch softmax collapse at 64 Chebyshev nodes -----------
        # e chunk broadcast via K=2 matmul; exp + weighted sums per chunk
        pn = p_cl.tile([128, G], F32, tag="ndB")
        zc = p_sm.tile([128, GC], F32, tag="zc")
        nmc = p_sm.tile([128, GC], F32, tag="nmc")
        for i in range(GC):
            sl = slice(i * 512, (i + 1) * 512)
            ep = ps_mm.tile([128, 512], F32, tag="mm", name=f"ep{i}")
            nc.tensor.matmul(ep[:], bh_sb[:], e2[:, sl], start=True, stop=True)
            nc.scalar.activation(pn[:, sl], ep[:], AF.Exp,
                                 scale=cst_sb[:, O_XN:O_XN + 1].bitcast(F32),
                                 accum_out=zc[:, i:i + 1])
            nc.vector.scalar_tensor_tensor(
                out=pn[:, sl], in0=pn[:, sl], scalar=1.0, in1=ep[:],
                op0=OP.mult, op1=OP.mult, accum_out=nmc[:, i:i + 1])
        z_col = p_sm.tile([128, 1], F32, tag="zcol")
        nc.vector.tensor_reduce(z_col[:], zc[:], axis=AX.X, op=OP.add)
        nm_col = p_sm.tile([128, 1], F32, tag="nmcol")
        nc.vector.tensor_reduce(nm_col[:], nmc[:], axis=AX.X, op=OP.add)
        zr_col = p_sm.tile([128, 1], F32, tag="zrc")
        nc.vector.reciprocal(zr_col[:], z_col[:])
        f_col = p_sm.tile([128, 1], F32, tag="fc")
        nc.vector.tensor_mul(f_col[:], nm_col[:], zr_col[:])
        if DBG:
            nc.sync.dma_start(dbg["d_f"][:], f_col[:])

        # ---- early LN1 stats from xs (y = xs + uv*w_h + cv is low-rank) --
        def pcell(kt):
            return pc[0:1, kt * NPC + 6:kt * NPC + 7]

        st0 = ps_x.tile([1, T], F32, tag="x", name="st0a")
        st1a = ps_x.tile([1, T], F32, tag="x", name="st1a")
        st1b = ps_x.tile([1, T], F32, tag="x", name="st1b")
        qu = ps_x.tile([H, T], F32, tag="x", name="qu")
        sqx_t = []
        for kt in range(KC):
            sqx = p_act.tile([128, T], FP16, tag="sqx", bufs=2, name=f"sqx{kt}")
            nc.scalar.activation(sqx[:], xs[:, kt * T:(kt + 1) * T].bitcast(F32),
                                 AF.Square)
            sqx_t.append(sqx)
        for kt in range(KC):
            nc.tensor.matmul(st1a[:], onesk_h[:], sqx_t[kt][:],
                             start=(kt == 0), stop=(kt == KC - 1))
        for kt in range(KC):
            nc.tensor.matmul(st0[:], pccol_r(kt, 3), xs[:, kt * T:(kt + 1) * T],
                             start=(kt == 0), stop=False)
        for kt in range(KC):
            nc.tensor.matmul(st1b[:], pccol_r(kt, 4), xs[:, kt * T:(kt + 1) * T],
                             start=(kt == 0), stop=False)
        nc.tensor.matmul(st1b[:], pcell(2), ones_row[:], start=False, stop=False)
        for kt in range(KC):
            nc.tensor.matmul(qu[:], v_sb[:, kt * H:(kt + 1) * H],
                             xs[:, kt * T:(kt + 1) * T],
                             start=(kt == 0), stop=(kt == KC - 1))

        # cb[p, k] = ck[batch(p), k] via block-ones matmul (no DRAM trip)
        fdct = p_sm.tile([128, KDEG], F32R, tag="fdct")
        nc.vector.tensor_scalar_mul(
            fdct[:], cst_sb[:, O_DCT:O_DCT + KDEG].bitcast(F32), f_col[:])
        pcb = ps_mm.tile([128, KDEG], F32, tag="mm", name="pcb")
        nc.tensor.matmul(pcb[:], cst_sb[:, O_BLK:O_BLK + 128], fdct[:],
                         start=True, stop=True)
        cb = p_cl.tile([128, KDEG], F32, tag="cb")
        nc.vector.tensor_copy(cb[:], pcb[:])
        if DBG:
            nc.sync.dma_start(dbg["d_tts"][:], tt_sb[:])
            nc.sync.dma_start(dbg["d_tt"][:], tt[:])
            nc.sync.dma_start(dbg["d_cb"][:], cb[:])

        # ---- Clenshaw over packed a: [128, 32] ---------------------------
        tt2 = p_cl.tile([128, FP], F32, tag="tt2")
        nc.vector.tensor_add(tt2[:], tt[:], tt[:])
        bb1 = p_cl.tile([128, FP], F32, tag="bb1")
        bb2 = p_cl.tile([128, FP], F32, tag="bb2")
        tmp = p_cl.tile([128, FP], F32, tag="tmp")
        nc.vector.memset(bb1[:], 0.0)
        nc.vector.memset(bb2[:], 0.0)
        cur1, cur2 = bb1, bb2
        wscr = p_sm.tile([2, FP], F32, tag="wscr")
        for k in range(KDEG - 1, 0, -1):
            nc.vector.tensor_mul(tmp[:], tt2[:], cur1[:])
            nc.vector.scalar_tensor_tensor(
                out=cur2[:], in0=tmp[:], scalar=cb[:, k:k + 1], in1=cur2[:],
                op0=OP.add, op1=OP.subtract)
            cur1, cur2 = cur2, cur1
            if k % 3 == 1:
                # HAM warm-keeper: trivial matmul touching the live Clenshaw
                # state so it is pinned inside this window (PE idles here
                # otherwise and the clock gate drops to 1.2 GHz for FFN1)
                pwm = ps_mm.tile([2, FP], F32, tag="mm", name=f"warm{k}")
                nc.tensor.matmul(pwm[:], cb[:, 0:2], cur1[:],
                                 start=True, stop=True)
                nc.vector.tensor_copy(wscr[:], pwm[:])
        nc.sync.dma_start(warm_d[:], wscr[:])
        w_pack = p_cl.tile([128, FP], F32R, tag="wp")
        nc.vector.tensor_mul(tmp[:], tt[:], cur1[:])
        nc.vector.scalar_tensor_tensor(
            out=w_pack[:], in0=tmp[:], scalar=cb[:, 0:1], in1=cur2[:],
            op0=OP.add, op1=OP.subtract)
        if DBG:
            nc.sync.dma_start(dbg["d_wp"][:], w_pack[:].bitcast(F32))

        # ---- w to [H, T] layout via selector matmuls ---------------------
        pw = ps_mm.tile([H, T], F32, tag="mm", name="pw")
        for b in range(B):
            for lhi in range(8):
                jb = b * 8 + lhi
                nc.tensor.matmul(
                    pw[:, b * LC + lhi * FP:b * LC + (lhi + 1) * FP],
                    cst_sb[:, O_RV + jb * H:O_RV + (jb + 1) * H],
                    w_pack[:], start=True, stop=True)
        w_HT = p_sm.tile([H, T], F32R, tag="wht")
        nc.vector.tensor_copy(w_HT[:], pw[:])
        if DBG:
            nc.sync.dma_start(dbg["d_wht"][:], w_HT[:].bitcast(F32))

        # ---- finish LN1 stats with the w-dependent low-rank terms --------
        # st0 += (su'/C) @ w_HT + (sum cv)/C * ones
        nc.tensor.matmul(st0[:], pccol_r(1, 5)[0:H, :], w_HT[:],
                         start=False, stop=False)
        nc.tensor.matmul(st0[:], pcell(1), ones_row[:], start=False, stop=True)
        wqu = p_sm.tile([H, T], F32R, tag="wqu")
        nc.vector.tensor_mul(wqu[:], w_HT[:].bitcast(F32), qu[:])
        wsq = p_sm.tile([H, T], F32R, tag="wsq")
        nc.vector.tensor_mul(wsq[:], w_HT[:].bitcast(F32), w_HT[:].bitcast(F32))
        nc.tensor.matmul(st1b[:], pccol_r(2, 5)[0:H, :], wqu[:],
                         start=False, stop=False)
        nc.tensor.matmul(st1b[:], pccol_r(3, 5)[0:H, :], wsq[:],
                         start=False, stop=False)
        nc.tensor.matmul(st1b[:], pccol_r(0, 6)[0:H, :], w_HT[:],
                         start=False, stop=True)

        # ---- x_attn + residual -> y via selector matmul ------------------
        y_t = []
        for mt in range(KC):
            wr = ps_mm.tile([128, T], F32, tag="mm", name=f"wr{mt}")
            nc.tensor.matmul(wr[:], sel_sb[:, mt * 128:(mt + 1) * 128],
                             w_HT[:], start=True, stop=False)
            nc.tensor.matmul(wr[:], rows_sb[0:1, mt * 128:(mt + 1) * 128],
                             ones_row[:], start=False, stop=True)
            yk = p_act.tile([128, T], FP16, tag="y", name=f"y{mt}")
            # yk = (wr * uv) + xs   (wr already contains w_bcast + cv)
            nc.vector.scalar_tensor_tensor(
                out=yk[:], in0=wr[:], scalar=pccol(mt, 1),
                in1=xs[:, mt * T:(mt + 1) * T].bitcast(F32),
                op0=OP.mult, op1=OP.add)
            y_t.append(yk)
        if DBG:
            for mt in range(KC):
                nc.gpsimd.dma_start(dbg["d_y"][:, mt * T:(mt + 1) * T],
                                    y_t[mt][:])

        def ln_rows(s0, s1a, s1b, ph):
            """stat psums -> (rstd_row F32R psum-broadcastable, q_row F32R)."""
            musq = p_sm.tile([1, T], F32, tag="lnrow", bufs=4, name=f"musq{ph}")
            nc.scalar.activation(musq[:], s0[:], AF.Square)
            var = p_sm.tile([1, T], F32, tag="lnrow", bufs=4, name=f"var{ph}")
            if s1b is not None:
                t1 = p_sm.tile([1, T], F32, tag="lnrow", bufs=4, name=f"t1{ph}")
                nc.vector.tensor_sub(t1[:], s1a[:], musq[:])
                nc.vector.tensor_add(var[:], t1[:], s1b[:])
            else:
                nc.vector.tensor_sub(var[:], s1a[:], musq[:])
            rstd_f32 = p_sm.tile([1, T], F32, tag="lnrow", bufs=4, name=f"rsf{ph}")
            nc.scalar.activation(rstd_f32[:], var[:], AF.Abs_reciprocal_sqrt,
                                 bias=eps_col[:])
            rstd_row = p_sm.tile([1, T], F32R, tag="lnrow", bufs=4,
                                 name=f"rstd{ph}")
            nc.vector.tensor_copy(rstd_row[:], rstd_f32[:])
            q_row = p_sm.tile([1, T], F32R, tag="lnrow", bufs=4, name=f"q{ph}")
            nc.vector.tensor_mul(q_row[:], s0[:], rstd_f32[:])
            return rstd_row, q_row

        def xt_tiles(y_tiles, rstd_row, ph):
            """x~ = y * rstd (per token) as fp16 tiles."""
            outs = []
            for kt in range(KC):
                pR = ps_mm.tile([128, T], F32, tag="mm", name=f"pR{ph}{kt}")
                nc.tensor.matmul(pR[:], ones_row[0:1, 0:128], rstd_row[:],
                                 start=True, stop=True)
                xo = p_act.tile([128, T], FP16, tag=f"ln{ph}", bufs=4,
                                name=f"ln{ph}{kt}")
                nc.vector.tensor_mul(xo[:], y_tiles[kt][:], pR[:])
                outs.append(xo)
            return outs

        rstd1, q1 = ln_rows(st0, st1a, st1b, "a")
        x_t = xt_tiles(y_t, rstd1, "a")
        if DBG:
            for mt in range(KC):
                nc.gpsimd.dma_start(dbg["d_x"][:, mt * T:(mt + 1) * T],
                                    x_t[mt][:])

        # ---- FFN1: h = relu(W1g @ x~ + r1*q1 + b1') ----------------------
        h_t = []
        for mt in range(KH):
            pf = ps_mm.tile([128, T], F32, tag="mm", name=f"pf1{mt}")
            for kt in range(KC):
                sl = slice(kt * 4 * C + mt * 128, kt * 4 * C + (mt + 1) * 128)
                nc.tensor.matmul(pf[:], w1_sb[:, sl], x_t[kt][:],
                                 start=(kt == 0), stop=False)
            nc.tensor.matmul(pf[:], rows_sb[0:1, 4 * C + mt * 128:
                                            4 * C + (mt + 1) * 128],
                             q1[:], start=False, stop=True)
            hm = p_big.tile([128, T], FP16, tag="h", bufs=16, name=f"h{mt}")
            nc.scalar.activation(hm[:], pf[:], AF.Relu, bias=b1c[:, mt:mt + 1])
            h_t.append(hm)

        # ---- FFN2 + residual: y2 = g1*x~ - g1*q1 + be1 + W2@h + b2 -------
        y2_t = []
        for mt in range(KC):
            pf = ps_mm.tile([128, T], F32, tag="mm", name=f"pf2{mt}")
            for kt in range(KH):
                sl = slice(kt * C + mt * 128, kt * C + (mt + 1) * 128)
                nc.tensor.matmul(pf[:], w2_sb[:, sl], h_t[kt][:],
                                 start=(kt == 0), stop=False)
            nc.tensor.matmul(pf[:], rows_sb[0:1, C + mt * 128:C + (mt + 1) * 128],
                             q1[:], start=False, stop=False)
            nc.tensor.matmul(pf[:],
                             rows_sb[0:1, 2 * C + mt * 128:2 * C + (mt + 1) * 128],
                             ones_row[:], start=False, stop=True)
            y2 = p_act.tile([128, T], FP16, tag="y", name=f"y2{mt}")
            nc.vector.scalar_tensor_tensor(
                out=y2[:], in0=x_t[mt][:], scalar=pccol(mt, 2),
                in1=pf[:], op0=OP.mult, op1=OP.add)
            y2_t.append(y2)

        # ---- LN2 stats (classic) -----------------------------------------
        st0b = ps_x.tile([1, T], F32, tag="x", name="st0b")
        st1c = ps_x.tile([1, T], F32, tag="x", name="st1c")
        for kt in range(KC):
            nc.tensor.matmul(st0b[:], onesk_h[:], y2_t[kt][:],
                             start=(kt == 0), stop=(kt == KC - 1))
        sq_t = []
        for kt in range(KC):
            sq = p_act.tile([128, T], FP16, tag="sqx", bufs=2, name=f"sqb{kt}")
            nc.scalar.activation(sq[:], y2_t[kt][:], AF.Square)
            sq_t.append(sq)
        for kt in range(KC):
            nc.tensor.matmul(st1c[:], onesk_h[:], sq_t[kt][:],
                             start=(kt == 0), stop=(kt == KC - 1))
        rstd2, q2 = ln_rows(st0b, st1c, None, "b")
        z_t = xt_tiles(y2_t, rstd2, "b")

        # ---- output: out = Wog2 @ z~ + r3*q2 + bo' -----------------------
        for mt in range(KC):
            pf = ps_mm.tile([128, T], F32, tag="mm", name=f"pfo{mt}")
            for kt in range(KC):
                sl = slice(kt * C + mt * 128, kt * C + (mt + 1) * 128)
                nc.tensor.matmul(pf[:], wo_sb[:, sl], z_t[kt][:],
                                 start=(kt == 0), stop=False)
            nc.tensor.matmul(pf[:],
                             rows_sb[0:1, 3 * C + mt * 128:3 * C + (mt + 1) * 128],
                             q2[:], start=False, stop=True)
            om = p_act.tile([128, T], F32, tag="tmpx", bufs=2, name=f"om{mt}")
            nc.scalar.activation(om[:], pf[:], AF.Identity, bias=pccol(mt, 0))
            nc.sync.dma_start(
                out_sl[:, mt * 128:(mt + 1) * 128, :].rearrange("b c l -> c b l"),
                om[:])

    nc.compile()
    return nc


def kernel(**inputs):
    global _CACHE, LAST_RESULTS
    if _CACHE is None:
        _CACHE = _build()
    nc = _CACHE

    f32 = lambda x: np.asarray(x, dtype=np.float32)
    f16t = lambda x: np.ascontiguousarray(np.asarray(x).T, dtype=np.float16)
    seq = f32(inputs["seq"])

    # host-side stage A: all weight-only precomputation (exact fp32 math)
    Wg = f32(inputs["Wg"])[:, 0]
    bg = f32(inputs["bg"])
    Wk, Wv, Wq = f32(inputs["Wk"]), f32(inputs["Wv"]), f32(inputs["Wq"])
    bq, bv = f32(inputs["bq"]), f32(inputs["bv"])
    g1, be1 = f32(inputs["g1"]), f32(inputs["beta1"])
    g2, be2 = f32(inputs["g2"]), f32(inputs["beta2"])
    W1, b1 = f32(inputs["W1"]), f32(inputs["b1"])
    W2, b2 = f32(inputs["W2"]), f32(inputs["b2"])
    Wo, bo = f32(inputs["Wo"]), f32(inputs["bo"])
    uk = Wk @ Wg
    uv = Wv @ Wg
    cv = Wv @ bg + bv
    mask = np.zeros((C, H), np.float32)
    for h in range(H):
        mask[h * D:(h + 1) * D, h] = 1.0
    U = mask * uk[:, None]
    V = mask * uv[:, None]
    M = (Wq.T @ U).astype(np.float32)                      # [C, H]
    a0s = ((U.T @ bq) * SCALE / SCAL).astype(np.float32)   # [H]

    # LN folds
    W1g = W1 * g1[None, :]
    Wog2 = Wo * g2[None, :]
    b1p = b1 + W1 @ be1
    bop = bo + Wo @ be2
    r1 = -W1g.sum(axis=1)                                  # [4C]
    r3 = -Wog2.sum(axis=1)                                 # [C]

    # per-kt packed columns
    miscA = np.zeros((KC, 128), np.float32)
    miscA[0, :H] = a0s
    miscA[1, :H] = (V.sum(axis=0)) / C                     # su'/C
    miscA[2, :H] = 2.0 / C
    miscA[3, :H] = ((uv * uv)[:, None] * mask).sum(axis=0) / C
    miscB = np.zeros((KC, 128), np.float32)
    miscB[0, :H] = 2.0 * ((uv * cv)[:, None] * mask).sum(axis=0) / C
    miscB[1, 0] = cv.sum() / C
    miscB[2, 0] = (cv * cv).sum() / C
    cols = [bop.reshape(KC, 128), uv.reshape(KC, 128), g1.reshape(KC, 128),
            np.full((KC, 128), 1.0 / C, np.float32),
            (2.0 * cv / C).reshape(KC, 128), miscA, miscB]
    pcd = np.ascontiguousarray(
        np.stack(cols, axis=2).transpose(1, 0, 2).reshape(128, KC * NPC))
    md = np.ascontiguousarray(
        M.reshape(KC, 128, H).transpose(1, 0, 2).reshape(128, KC * H))
    vd_img = np.ascontiguousarray(
        V.reshape(KC, 128, H).transpose(1, 0, 2).reshape(128, KC * H))
    b1d = np.ascontiguousarray(b1p.reshape(KH, 128).T)
    rowsd = np.ascontiguousarray(np.concatenate(
        [cv, -g1, be1 + b2, r3, r1]).reshape(1, 8 * C))

    def wimg(wT_f16, ksplit):
        # [Cin, Cout] -> [128, ksplit*Cout] image (partition-folded)
        cin, cout = wT_f16.shape
        return np.ascontiguousarray(
            wT_f16.reshape(ksplit, 128, cout).transpose(1, 0, 2).reshape(
                128, ksplit * cout))

    f16 = lambda x: np.ascontiguousarray(x.T, dtype=np.float16)
    base = {
        "expv": f32(inputs["exp"]),
        "w1t": wimg(f16(W1g), KC),
        "w2t": wimg(f16t(inputs["W2"]), KH),
        "wot": wimg(f16(Wog2), KC),
        "pcd": pcd,
        "md": md,
        "vd": vd_img,
        "b1d": b1d,
        "rowsd": rowsd,
    }
    # xs image per core: [128, kt*T + b*LC + l] = seq[b, kt*128+p, c0+l]
    seq_r = seq.reshape(B, KC, 128, L)
    in_maps = []
    for c in range(NCORES):
        m = dict(base)
        sl = seq_r[:, :, :, c * LC:(c + 1) * LC]           # [B, KC, 128, LC]
        m["seq_sl"] = np.ascontiguousarray(
            sl.transpose(2, 1, 0, 3).reshape(128, KC * T))
        in_maps.append(m)

    res = run_bass_kernel_spmd(nc, in_maps, list(range(NCORES)), trace=TRACE,
                               **TRACE_KW)
    LAST_RESULTS = res
    out = np.empty((B, C, L), np.float32)
    for c in range(NCORES):
        out[:, :, c * LC:(c + 1) * LC] = res.results[c]["out_sl"]
    return out
